# revision 12
# baseline (speedup 1.0000x reference)
"""Multi-head attention (B=2, S=4096, D=512, H=8) on 8 TRN2 NeuronCores.

Sharding: core c handles batch b=c//4 and head-pair hg=c%4 (channels
cb=hg*128 .. cb+128). Each core computes its 2 heads' attention and a
partial output projection (rows of Wo); host sums the 4 partials per batch.

Device kernel (per core, all layouts chosen so no on-device transpose of
activations is needed; host passes x^T):
  qh_T/kh_T [128ch, S]  = W_slice @ x^T        (PE, fp32, cast to bf16)
  vh        [S, 128ch]  natural layout, with a ones column appended per head
  scores_T  [kv, sq]    = kh_T^T-slices @ qh_T (PE, bf16, K=64 row-packed)
  p = exp(scores_T)                            (ACT, PSUM->SBUF bf16)
  ctx_T|l   = [vh|1]^T @ p                     (PE; row 64 = softmax denom)
  out_part  = sum_h (ctx_h @ WoT_h) * (1/l_h)  (PE + DVE per-row scaling)
"""

from contextlib import ExitStack

import numpy as np

import concourse.bass as bass
import concourse.mybir as mybir
import concourse.tile as tile
from concourse import bacc, bass_utils

S = 4096
DM = 512
DK = 64
HPC = 2  # heads per core
CB = HPC * DK  # 128 channel block per core
KC = 4  # contraction chunks of 128 over DM
JB = 1024  # S_q block width
NJ = S // JB
NKV = S // 128  # 32 kv tiles
FP32 = mybir.dt.float32
BF16 = mybir.dt.bfloat16

_CACHE = {}


def _build(trace_scopes=False):
    nc = bacc.Bacc("TRN2", target_bir_lowering=False, debug=False)

    xqT = nc.dram_tensor("xqT", [KC, 128, S], FP32, kind="ExternalInput")
    xkT = nc.dram_tensor("xkT", [KC, 128, S], FP32, kind="ExternalInput")
    xvT = nc.dram_tensor("xvT", [KC, 128, S], FP32, kind="ExternalInput")
    wq = nc.dram_tensor("wq", [128, KC, CB], FP32, kind="ExternalInput")
    wk = nc.dram_tensor("wk", [128, KC, CB], FP32, kind="ExternalInput")
    wv = nc.dram_tensor("wv", [128, KC, CB], FP32, kind="ExternalInput")
    woT = nc.dram_tensor("woT", [CB, DM], BF16, kind="ExternalInput")
    outp = nc.dram_tensor("outp", [S, DM], FP32, kind="ExternalOutput")

    with tile.TileContext(nc) as tc, ExitStack() as ctx:
        singles = ctx.enter_context(tc.tile_pool(name="singles", bufs=1))
        xpool = ctx.enter_context(tc.tile_pool(name="xpool", bufs=2))
        ppool = ctx.enter_context(tc.tile_pool(name="ppool", bufs=4))
        opool = ctx.enter_context(tc.tile_pool(name="opool", bufs=3))
        tpool = ctx.enter_context(tc.tile_pool(name="tpool", bufs=4))


        # --- persistent sbuf state -----------------------------------------
        wq_sb = singles.tile([128, KC, CB], FP32)
        wk_sb = singles.tile([128, KC, CB], FP32)
        wv_sb = singles.tile([128, KC, CB], FP32)
        woT_sb = singles.tile([CB, DM], BF16)
        nc.sync.dma_start(out=wq_sb, in_=wq[:, :, :])
        nc.sync.dma_start(out=wk_sb, in_=wk[:, :, :])
        nc.sync.dma_start(out=wv_sb, in_=wv[:, :, :])
        nc.sync.dma_start(out=woT_sb, in_=woT[:, :])

        qh_sb = singles.tile([CB, S], BF16)  # rows h*64.. = head h, scaled 1/8
        kh_sb = singles.tile([CB, S], BF16)
        vh_sb = singles.tile([128, NKV, 2 * (DK + 1)], BF16)  # [.., h*65:+64]=vh, col h*65+64=1
        ctx2_sb = singles.tile([CB, S], BF16)  # unnormalized ctx_T
        recip_sb = singles.tile([128, 2, NKV], FP32)  # 1/l per (head, seq-tile)
        ones1 = singles.tile([1, 1], FP32)
        nc.vector.memset(ones1, 1.0)
        for h in range(HPC):
            nc.vector.memset(vh_sb[:, :, h * (DK + 1) + DK], 1.0)

        # --- phase A: projections ------------------------------------------
        scale = 1.0 / np.sqrt(DK)
        a_ctx = ExitStack()
        ps_a = a_ctx.enter_context(tc.tile_pool(name="ps_a", bufs=2, space="PSUM"))
        for sb in range(S // 512):
            sl = slice(sb * 512, (sb + 1) * 512)
            xq_t = [xpool.tile([128, 512], FP32, tag=f"xq{kc}", name=f"xq{kc}")
                    for kc in range(KC)]
            xk_t = [xpool.tile([128, 512], FP32, tag=f"xk{kc}", name=f"xk{kc}")
                    for kc in range(KC)]
            xv_t = [xpool.tile([128, 512], FP32, tag=f"xv{kc}", name=f"xv{kc}")
                    for kc in range(KC)]
            for kc in range(KC):
                nc.sync.dma_start(out=xq_t[kc], in_=xqT[kc, :, sl])
                nc.sync.dma_start(out=xk_t[kc], in_=xkT[kc, :, sl])
                nc.sync.dma_start(out=xv_t[kc], in_=xvT[kc, :, sl])
            q_ps = ps_a.tile([128, 512], FP32, tag="qk")
            for kc in range(KC):
                nc.tensor.matmul(q_ps[:CB], wq_sb[:, kc, :], xq_t[kc],
                                 start=(kc == 0), stop=(kc == KC - 1))
            nc.scalar.activation(qh_sb[:, sl], q_ps[:CB],
                                 mybir.ActivationFunctionType.Copy, scale=scale)
            k_ps = ps_a.tile([128, 512], FP32, tag="qk")
            for kc in range(KC):
                nc.tensor.matmul(k_ps[:CB], wk_sb[:, kc, :], xk_t[kc],
                                 start=(kc == 0), stop=(kc == KC - 1))
            nc.vector.tensor_copy(kh_sb[:, sl], k_ps[:CB])
            for st in range(4):
                t = sb * 4 + st
                v_ps = ps_a.tile([128, CB], FP32, tag="v")
                for kc in range(KC):
                    nc.tensor.matmul(v_ps, xv_t[kc][:, st * 128:(st + 1) * 128],
                                     wv_sb[:, kc, :],
                                     start=(kc == 0), stop=(kc == KC - 1))
                nc.scalar.copy(vh_sb[:, t, 0:DK], v_ps[:, 0:DK])
                nc.vector.tensor_copy(vh_sb[:, t, DK + 1:2 * DK + 1], v_ps[:, DK:CB])

        a_ctx.close()
        ps_sc = ctx.enter_context(tc.tile_pool(name="ps_sc", bufs=2, space="PSUM"))
        ps_cx = ctx.enter_context(tc.tile_pool(name="ps_cx", bufs=1, space="PSUM"))

        # --- phase B: attention; phase C: output projection ----------------
        for j in range(NJ):
            jsl = slice(j * JB, (j + 1) * JB)
            cx = [ps_cx.tile([128, JB], FP32, tag=f"cx{h}", name=f"cx{h}")
                  for h in range(HPC)]
            for i in range(NKV):
                isl = slice(i * 128, (i + 1) * 128)
                for h in range(HPC):
                    hsl = slice(h * DK, (h + 1) * DK)
                    sc = ps_sc.tile([128, JB], FP32, tag="sc")
                    for half in range(JB // 512):
                        qsl = slice(j * JB + half * 512, j * JB + (half + 1) * 512)
                        nc.tensor.matmul(sc[:, half * 512:(half + 1) * 512],
                                         kh_sb[hsl, isl], qh_sb[hsl, qsl],
                                         start=True, stop=True)
                    p_t = ppool.tile([128, JB], BF16, tag="p")
                    nc.scalar.activation(p_t, sc, mybir.ActivationFunctionType.Exp)
                    vsl = slice(h * (DK + 1), (h + 1) * (DK + 1))
                    for half in range(JB // 512):
                        nc.tensor.matmul(
                            cx[h][:DK + 1, half * 512:(half + 1) * 512],
                            vh_sb[:, i, vsl], p_t[:, half * 512:(half + 1) * 512],
                            start=(i == 0), stop=(i == NKV - 1))
            # drain ctx for this j
            for h in range(HPC):
                nc.vector.tensor_copy(ctx2_sb[h * DK:(h + 1) * DK, jsl], cx[h][:DK])
                l_sb = tpool.tile([1, JB], FP32, tag="l")
                nc.vector.tensor_copy(l_sb, cx[h][DK:DK + 1])
                lt_ps = ps_sc.tile([128, 8], FP32, tag="sc")
                for st in range(8):
                    nc.tensor.matmul(lt_ps[:, st:st + 1],
                                     l_sb[:, st * 128:(st + 1) * 128], ones1,
                                     start=True, stop=True)
                nc.vector.reciprocal(recip_sb[:, h, j * 8:(j + 1) * 8], lt_ps)
            # phase C for the 8 seq tiles of this j
            for st in range(8):
                t = j * 8 + st
                tsl = slice(t * 128, (t + 1) * 128)
                po = []
                for h in range(HPC):
                    hsl = slice(h * DK, (h + 1) * DK)
                    po_ps = ps_sc.tile([128, DM], FP32, tag="sc")
                    nc.tensor.matmul(po_ps, ctx2_sb[hsl, tsl], woT_sb[hsl, :],
                                     start=True, stop=True)
                    po.append(po_ps)
                tmp0 = tpool.tile([128, DM], FP32, tag="tmp0")
                tmp1 = tpool.tile([128, DM], FP32, tag="tmp1")
                nc.vector.tensor_scalar_mul(tmp0, po[0], recip_sb[:, 0, t:t + 1])
                nc.vector.tensor_scalar_mul(tmp1, po[1], recip_sb[:, 1, t:t + 1])
                o_t = opool.tile([128, DM], FP32, tag="o")
                nc.vector.tensor_add(o_t, tmp0, tmp1)
                nc.sync.dma_start(out=outp[tsl, :], in_=o_t)
    nc.compile()
    return nc


def _get_nc():
    if "nc" not in _CACHE:
        _CACHE["nc"] = _build()
    return _CACHE["nc"]


def make_in_maps(q, k, v, Wq, Wk, Wv, Wo):
    import ml_dtypes
    q = np.asarray(q, np.float32)
    k = np.asarray(k, np.float32)
    v = np.asarray(v, np.float32)
    xT = {}
    for b in range(2):
        xT[("q", b)] = np.ascontiguousarray(q[b].T).reshape(KC, 128, S)
        xT[("k", b)] = np.ascontiguousarray(k[b].T).reshape(KC, 128, S)
        xT[("v", b)] = np.ascontiguousarray(v[b].T).reshape(KC, 128, S)
    in_maps = []
    for c in range(8):
        b, hg = divmod(c, 4)
        cb = hg * CB
        wq_c = np.ascontiguousarray(
            np.asarray(Wq, np.float32)[cb:cb + CB, :].T.reshape(KC, 128, CB)
            .transpose(1, 0, 2))
        wk_c = np.ascontiguousarray(
            np.asarray(Wk, np.float32)[cb:cb + CB, :].T.reshape(KC, 128, CB)
            .transpose(1, 0, 2))
        wv_c = np.ascontiguousarray(
            np.asarray(Wv, np.float32)[cb:cb + CB, :].T.reshape(KC, 128, CB)
            .transpose(1, 0, 2))
        woT_c = np.ascontiguousarray(np.asarray(Wo, np.float32)[:, cb:cb + CB].T)
        in_maps.append(dict(
            xqT=xT[("q", b)], xkT=xT[("k", b)], xvT=xT[("v", b)],
            wq=wq_c, wk=wk_c, wv=wv_c,
            woT=woT_c.astype(ml_dtypes.bfloat16),
        ))
    return in_maps


def kernel(q, k, v, Wq, bq, Wk, bk, Wv, bv, Wo, bo):
    nc = _get_nc()
    in_maps = make_in_maps(q, k, v, Wq, Wk, Wv, Wo)
    res = bass_utils.run_bass_kernel_spmd(nc, in_maps, core_ids=list(range(8)))
    parts = [r["outp"] for r in res.results]
    out = np.stack([parts[0] + parts[1] + parts[2] + parts[3],
                    parts[4] + parts[5] + parts[6] + parts[7]])
    out += np.asarray(bo, np.float32)[None, None, :]
    return out.astype(np.float32)


# revision 20
# speedup vs baseline: 1.1319x; 1.1319x over previous
"""Multi-head attention (B=2, S=4096, D=512, H=8) on 8 TRN2 NeuronCores.

Sharding: core c handles batch b=c//4 and head-pair hg=c%4 (channels
cb=hg*128 .. cb+128). Each core computes its 2 heads' attention and a
partial output projection (rows of Wo); host sums the 4 partials per batch.

Device kernel (per core, all layouts chosen so no on-device transpose of
activations is needed; host passes x^T):
  qh_T/kh_T [128ch, S]  = W_slice @ x^T        (PE, f32r, cast to bf16)
  vh        [S, 128ch]  natural layout, with a ones column appended per head
  scores_T  [kv, sq]    = kh_T^T-slices @ qh_T (PE, bf16, K=64 row-packed)
  p = exp(scores_T)                            (ACT, PSUM->SBUF bf16)
  ctx_T|l   = [vh|1]^T @ p                     (PE; row 64 = softmax denom)
  out_part  = sum_h (ctx_h @ WoT_h) * (1/l_h)  (PE + DVE per-row scaling)
Output projection for block j is interleaved into block j+1's attention to
keep PE/DVE busy while ACT (the bottleneck) streams exps.
"""

from contextlib import ExitStack

import numpy as np

import concourse.bass as bass
import concourse.mybir as mybir
import concourse.tile as tile
from concourse import bacc, bass_utils

S = 4096
DM = 512
DK = 64
HPC = 2  # heads per core
CB = HPC * DK  # 128 channel block per core
KC = 4  # contraction chunks of 128 over DM
JB = 1024  # S_q block width
NJ = S // JB
NKV = S // 128  # 32 kv tiles
FP32 = mybir.dt.float32
FP32R = mybir.dt.float32r
BF16 = mybir.dt.bfloat16

_CACHE = {}


def _build():
    nc = bacc.Bacc("TRN2", target_bir_lowering=False, debug=False)

    xqT = nc.dram_tensor("xqT", [KC, 128, S], FP32R, kind="ExternalInput")
    xkT = nc.dram_tensor("xkT", [KC, 128, S], FP32R, kind="ExternalInput")
    xvT = nc.dram_tensor("xvT", [KC, 128, S], FP32R, kind="ExternalInput")
    wq = nc.dram_tensor("wq", [128, KC, CB], FP32R, kind="ExternalInput")
    wk = nc.dram_tensor("wk", [128, KC, CB], FP32R, kind="ExternalInput")
    wv = nc.dram_tensor("wv", [128, KC, CB], FP32R, kind="ExternalInput")
    woT = nc.dram_tensor("woT", [CB, DM], BF16, kind="ExternalInput")
    outp = nc.dram_tensor("outp", [S, DM], FP32, kind="ExternalOutput")

    with tile.TileContext(nc) as tc, ExitStack() as ctx:
        singles = ctx.enter_context(tc.tile_pool(name="singles", bufs=1))
        xpool = ctx.enter_context(tc.tile_pool(name="xpool", bufs=2))
        ppool = ctx.enter_context(tc.tile_pool(name="ppool", bufs=6))
        opool = ctx.enter_context(tc.tile_pool(name="opool", bufs=3))
        tpool = ctx.enter_context(tc.tile_pool(name="tpool", bufs=4))
        ps = ctx.enter_context(tc.tile_pool(name="ps", bufs=1, space="PSUM"))

        # --- persistent sbuf state -----------------------------------------
        wq_sb = singles.tile([128, KC, CB], FP32R)
        wk_sb = singles.tile([128, KC, CB], FP32R)
        wv_sb = singles.tile([128, KC, CB], FP32R)
        woT_sb = singles.tile([CB, DM], BF16)
        nc.sync.dma_start(out=wq_sb, in_=wq[:, :, :])
        nc.sync.dma_start(out=wk_sb, in_=wk[:, :, :])
        nc.sync.dma_start(out=wv_sb, in_=wv[:, :, :])
        nc.sync.dma_start(out=woT_sb, in_=woT[:, :])

        qh_sb = singles.tile([CB, S], BF16)  # rows h*64.. = head h, scaled 1/8
        kh_sb = singles.tile([CB, S], BF16)
        vh_sb = singles.tile([128, NKV, 2 * (DK + 1)], BF16)  # col h*65+64 = 1.0
        ctx2_sb = singles.tile([CB, S], BF16)  # unnormalized ctx_T
        recip_sb = singles.tile([128, 2, NKV], FP32)  # 1/l per (head, seq-tile)
        ones1 = singles.tile([1, 1], FP32)
        nc.vector.memset(ones1, 1.0)
        for h in range(HPC):
            nc.vector.memset(vh_sb[:, :, h * (DK + 1) + DK], 1.0)

        def sc_tile(name):
            return ps.tile([128, JB], FP32, tag="sc", bufs=2, name=name)

        # --- phase A: projections (block sb); A(0) runs up front, A(1..3)
        # are interleaved into j=0's i-loop so ACT starts exping early -----
        scale = 1.0 / np.sqrt(DK)

        def a_dma(sb):
            sl = slice(sb * JB, (sb + 1) * JB)
            xq_t = [xpool.tile([128, JB], FP32R, tag=f"xq{kc}", name=f"xq{kc}")
                    for kc in range(KC)]
            xk_t = [xpool.tile([128, JB], FP32R, tag=f"xk{kc}", name=f"xk{kc}")
                    for kc in range(KC)]
            xv_t = [xpool.tile([128, JB], FP32R, tag=f"xv{kc}", name=f"xv{kc}")
                    for kc in range(KC)]
            for kc in range(KC):
                nc.sync.dma_start(out=xq_t[kc], in_=xqT[kc, :, sl])
                nc.sync.dma_start(out=xk_t[kc], in_=xkT[kc, :, sl])
                nc.sync.dma_start(out=xv_t[kc], in_=xvT[kc, :, sl])
            return xq_t, xk_t, xv_t

        def a_kq(sb, tiles, half):
            sl = slice(sb * JB + half * 512, sb * JB + (half + 1) * 512)
            hs = slice(half * 512, (half + 1) * 512)
            xq_t, xk_t, _ = tiles
            k_ps = sc_tile("k_ps")
            for kc in range(KC):
                nc.tensor.matmul(k_ps[:CB, :512], wk_sb[:, kc, :],
                                 xk_t[kc][:, hs],
                                 start=(kc == 0), stop=(kc == KC - 1))
            nc.vector.tensor_copy(kh_sb[:, sl], k_ps[:CB, :512])
            q_ps = sc_tile("q_ps")
            for kc in range(KC):
                nc.tensor.matmul(q_ps[:CB, :512], wq_sb[:, kc, :],
                                 xq_t[kc][:, hs],
                                 start=(kc == 0), stop=(kc == KC - 1))
            nc.vector.tensor_scalar_mul(qh_sb[:, sl], q_ps[:CB, :512], scale)

        def a_v(sb, tiles, group, v_tag_cx):
            xv_t = tiles[2]
            for st in range(group * 4, group * 4 + 4):
                t = sb * (JB // 128) + st
                if v_tag_cx:
                    v_ps = ps.tile([128, CB], FP32, tag=f"cx{st % 2}", bufs=1,
                                   name=f"v_ps{st % 2}")
                else:
                    v_ps = ps.tile([128, CB], FP32, tag="sc", bufs=2,
                                   name="v_ps")
                for kc in range(KC):
                    nc.tensor.matmul(v_ps, xv_t[kc][:, st * 128:(st + 1) * 128],
                                     wv_sb[:, kc, :],
                                     start=(kc == 0), stop=(kc == KC - 1))
                nc.vector.tensor_copy(vh_sb[:, t, 0:DK], v_ps[:, 0:DK])
                nc.vector.tensor_copy(vh_sb[:, t, DK + 1:2 * DK + 1],
                                      v_ps[:, DK:CB])

        def a_work(sb, v_tag_cx):
            tiles = a_dma(sb)
            for half in range(2):
                a_kq(sb, tiles, half)
            for g in range(2):
                a_v(sb, tiles, g, v_tag_cx)

        a_work(0, v_tag_cx=True)

        # --- phase B: attention; phase C interleaved one j behind ----------
        def c_work(t):
            tsl = slice(t * 128, (t + 1) * 128)
            po = []
            for h in range(HPC):
                hsl = slice(h * DK, (h + 1) * DK)
                po_ps = ps.tile([128, DM], FP32, tag="sc", bufs=2, name=f"po{h}")
                nc.tensor.matmul(po_ps, ctx2_sb[hsl, tsl], woT_sb[hsl, :],
                                 start=True, stop=True)
                po.append(po_ps)
            tmp0 = tpool.tile([128, DM], FP32, tag="tmp0")
            tmp1 = tpool.tile([128, DM], FP32, tag="tmp1")
            nc.vector.tensor_scalar_mul(tmp0, po[0], recip_sb[:, 0, t:t + 1])
            nc.vector.tensor_scalar_mul(tmp1, po[1], recip_sb[:, 1, t:t + 1])
            o_t = opool.tile([128, DM], FP32, tag="o")
            nc.vector.tensor_add(o_t, tmp0, tmp1)
            nc.sync.dma_start(out=outp[tsl, :], in_=o_t)

        def drain(j, cx):
            jsl = slice(j * JB, (j + 1) * JB)
            for h in range(HPC):
                nc.vector.tensor_copy(ctx2_sb[h * DK:(h + 1) * DK, jsl], cx[h][:DK])
                l_sb = tpool.tile([1, JB], FP32, tag="l")
                nc.vector.tensor_copy(l_sb, cx[h][DK:DK + 1])
                lt_ps = ps.tile([128, 8], FP32, tag="sc", bufs=2, name=f"lt{h}")
                for st in range(8):
                    nc.tensor.matmul(lt_ps[:, st:st + 1],
                                     l_sb[:, st * 128:(st + 1) * 128], ones1,
                                     start=True, stop=True)
                nc.vector.reciprocal(recip_sb[:, h, j * 8:(j + 1) * 8], lt_ps)

        prev_cx = None
        a_tiles = None
        for j in range(NJ):
            cx = [ps.tile([128, JB], FP32, tag=f"cx{h}", bufs=1, name=f"cx{h}")
                  for h in range(HPC)]
            for i in range(NKV):
                isl = slice(i * 128, (i + 1) * 128)
                for h in range(HPC):
                    hsl = slice(h * DK, (h + 1) * DK)
                    sc = sc_tile("sc")
                    for half in range(JB // 512):
                        qsl = slice(j * JB + half * 512, j * JB + (half + 1) * 512)
                        nc.tensor.matmul(sc[:, half * 512:(half + 1) * 512],
                                         kh_sb[hsl, isl], qh_sb[hsl, qsl],
                                         start=True, stop=True)
                    p_t = ppool.tile([128, JB], BF16, tag="p")
                    nc.scalar.activation(p_t, sc, mybir.ActivationFunctionType.Exp)
                    vsl = slice(h * (DK + 1), (h + 1) * (DK + 1))
                    for half in range(JB // 512):
                        nc.tensor.matmul(
                            cx[h][:DK + 1, half * 512:(half + 1) * 512],
                            vh_sb[:, i, vsl], p_t[:, half * 512:(half + 1) * 512],
                            start=(i == 0), stop=(i == NKV - 1))
                # drain previous j (deferred so j's exps start without a stall)
                if i == 1 and prev_cx is not None:
                    drain(j - 1, prev_cx)
                # interleave remaining projection blocks across j=0, quartered
                if j == 0 and i % 2 == 0 and i < 24:
                    sb = i // 8 + 1
                    part = (i % 8) // 2
                    if part == 0:
                        a_tiles = a_dma(sb)
                    if part < 2:
                        a_kq(sb, a_tiles, part)
                    else:
                        a_v(sb, a_tiles, part - 2, v_tag_cx=False)
                # interleave previous j's output projection, spread over i
                if j > 0 and i % 4 == 3:
                    c_work((j - 1) * 8 + i // 4)
            prev_cx = cx
        # tail: final drain + output projection for the last j block
        drain(NJ - 1, prev_cx)
        for st in range(8):
            c_work((NJ - 1) * 8 + st)
    nc.compile()
    return nc


def _get_nc():
    if "nc" not in _CACHE:
        _CACHE["nc"] = _build()
    return _CACHE["nc"]


def make_in_maps(q, k, v, Wq, Wk, Wv, Wo):
    import ml_dtypes
    q = np.asarray(q, np.float32)
    k = np.asarray(k, np.float32)
    v = np.asarray(v, np.float32)
    xT = {}
    for b in range(2):
        xT[("q", b)] = np.ascontiguousarray(q[b].T).reshape(KC, 128, S)
        xT[("k", b)] = np.ascontiguousarray(k[b].T).reshape(KC, 128, S)
        xT[("v", b)] = np.ascontiguousarray(v[b].T).reshape(KC, 128, S)
    in_maps = []
    for c in range(8):
        b, hg = divmod(c, 4)
        cb = hg * CB
        wq_c = np.ascontiguousarray(
            np.asarray(Wq, np.float32)[cb:cb + CB, :].T.reshape(KC, 128, CB)
            .transpose(1, 0, 2))
        wk_c = np.ascontiguousarray(
            np.asarray(Wk, np.float32)[cb:cb + CB, :].T.reshape(KC, 128, CB)
            .transpose(1, 0, 2))
        wv_c = np.ascontiguousarray(
            np.asarray(Wv, np.float32)[cb:cb + CB, :].T.reshape(KC, 128, CB)
            .transpose(1, 0, 2))
        woT_c = np.ascontiguousarray(np.asarray(Wo, np.float32)[:, cb:cb + CB].T)
        in_maps.append(dict(
            xqT=xT[("q", b)], xkT=xT[("k", b)], xvT=xT[("v", b)],
            wq=wq_c, wk=wk_c, wv=wv_c,
            woT=woT_c.astype(ml_dtypes.bfloat16),
        ))
    return in_maps


def kernel(q, k, v, Wq, bq, Wk, bk, Wv, bv, Wo, bo):
    nc = _get_nc()
    in_maps = make_in_maps(q, k, v, Wq, Wk, Wv, Wo)
    res = bass_utils.run_bass_kernel_spmd(nc, in_maps, core_ids=list(range(8)))
    parts = [r["outp"] for r in res.results]
    out = np.stack([parts[0] + parts[1] + parts[2] + parts[3],
                    parts[4] + parts[5] + parts[6] + parts[7]])
    out += np.asarray(bo, np.float32)[None, None, :]
    return out.astype(np.float32)
